# revision 23
# baseline (speedup 1.0000x reference)
"""DeepseekV3 MoE layer on 8 Trainium2 NeuronCores (Bass/Tile).

Sharding:
  - Router: data-parallel (each core routes its own T/8=512 tokens, fp32,
    selection done on exact logits), then AllGather of per-token routed
    weights [T, E] (w>0 encodes selection) -> every core knows the routing.
  - Capacity ranks: per-expert running count over tokens via one DVE prefix
    scan; rank <= C survives (matches the reference's stable-sort capacity
    drop).
  - Routed experts: expert-parallel, 4 experts/core, bf16 GEMMs (router
    stays fp32).  Token rows are dma_gather'ed (transpose mode) straight
    into the [D-part, slot] layout, GEMM'd on the real capacity (160 of the
    256 padded slots), weighted, and dma_scatter_add'ed into bf16 [T, D/2]
    column-half partials.
  - Combine: two ReduceScatters (add, bf16), one per column half, so the
    first overlaps the second half's down-projection; each core adds its
    locally computed shared-expert MLP (kept in SBUF) and writes the fp32
    output slice.

Weights are pre-blocked host-side so every streaming DMA is a contiguous
[128, N] 2D transfer.  Small/bookkeeping DMAs ride the Activation HWDGE,
weight streams ride the SP HWDGE.

kernel(**inputs) takes the full unsharded inputs and returns the full
[B, S, D] output.  Self-contained: hardcodes all shapes.
"""

import os
import sys

for _p in ("/opt/trn_rl_repo", "/opt/pypackages"):
    if _p not in sys.path:
        sys.path.insert(0, _p)

import numpy as np

# ---------------------------------------------------------------- constants
B, S, D = 2, 2048, 2048
T = B * S                  # 4096 tokens
I = 1024                   # routed expert intermediate
E = 32                     # routed experts
K = 4                      # experts per token
NG = 8                     # groups
GS = E // NG               # experts per group = 4
TKG = 3                    # top-k groups
ISH = 2048                 # shared expert intermediate (I * n_shared)
SCALE = 2.5
C = 160                    # capacity = ceil(1.25 * T / E)
CP = 256                   # per-expert slot padding (128-aligned)
NCORES = 8
EL = E // NCORES           # local experts per core = 4
TL = T // NCORES           # local tokens per core = 512

# "bf16" | "f32" | "f32r" : dtype/mode of the heavy GEMMs (router stays f32)
GEMM_MODE = os.environ.get("BASS_MOE_GEMM_MODE", "bf16")
# ablation: 1 -> replace collectives with local copies (WRONG results;
# isolates the HW collective cost)
NOCC = bool(int(os.environ.get("BASS_MOE_NOCC", "0")))

DC = D // 128            # 16 d-chunks
IC = I // 128            # 8  i-chunks
MC = ISH // 128          # 16 shared-intermediate chunks
DC4 = D // 512           # 4  512-wide d-chunks
DH = D // 2              # column half width
TT = TL // 128           # 4 own-token tiles
NT = T // 128            # 32 all-token tiles


# ---------------------------------------------------------------- builder
def _build(gemm_mode: str):
    import concourse.bass as bass
    import concourse.bacc as bacc
    import concourse.mybir as mybir
    import concourse.tile as tile
    from concourse import masks
    from contextlib import ExitStack

    dt = mybir.dt
    Alu = mybir.AluOpType
    Act = mybir.ActivationFunctionType

    f32 = dt.float32
    bf16 = dt.bfloat16
    wdt = bf16 if gemm_mode == "bf16" else f32

    def mm_cast(ap):
        if gemm_mode == "f32r":
            return ap.bitcast(dt.float32r)
        return ap

    nc = bacc.Bacc(None, num_devices=NCORES, num_swdge_queues=2)
    groups = [list(range(NCORES))]

    # ---------------- I/O ----------------
    x_full = nc.dram_tensor("x_full", [T, D], wdt, kind="ExternalInput")
    x_own = nc.dram_tensor("x_own", [TL, D], f32, kind="ExternalInput")
    rwT = nc.dram_tensor("rwT", [128, DC, E], f32, kind="ExternalInput")
    ebias = nc.dram_tensor("ebias", [1, E], f32, kind="ExternalInput")
    sloc = nc.dram_tensor("sloc", [E, EL], f32, kind="ExternalInput")
    wgu = nc.dram_tensor("wgu", [EL, IC, 128, 2 * DC * 128], wdt,
                         kind="ExternalInput")
    wd = nc.dram_tensor("wd", [EL, DC4, 128, IC * 512], wdt,
                        kind="ExternalInput")
    sgu = nc.dram_tensor("sgu", [MC, 128, 2 * DC * 128], wdt,
                         kind="ExternalInput")
    sd = nc.dram_tensor("sd", [DC4, 128, MC * 512], wdt, kind="ExternalInput")
    out = nc.dram_tensor("out", [TL, D], f32, kind="ExternalOutput")

    # ---------------- internal DRAM ----------------
    selw_own = nc.dram_tensor("selw_own", [TL, E], f32)
    selw_all = nc.dram_tensor("selw_all", [T, E], f32, addr_space="Shared")
    partial0 = nc.dram_tensor("partial0", [T, DH], bf16)
    partial1 = nc.dram_tensor("partial1", [T, DH], bf16)
    rs_out0 = nc.dram_tensor("rs_out0", [TL, DH], bf16)
    rs_out1 = nc.dram_tensor("rs_out1", [TL, DH], bf16)
    idx_dram = nc.dram_tensor("idx_dram", [16, EL * 16], dt.int16)
    at_dram = nc.dram_tensor("at_dram", [EL, T], f32)
    nf_dram = nc.dram_tensor("nf_dram", [1, EL], f32)
    aw_dram = nc.dram_tensor("aw_dram", [EL, T], f32)

    partials = [partial0, partial1]
    rs_outs = [rs_out0, rs_out1]

    with tile.TileContext(nc) as tc, ExitStack() as ctx:
        consts = ctx.enter_context(tc.tile_pool(name="consts", bufs=1))
        work = ctx.enter_context(tc.tile_pool(name="work", bufs=2))
        psum_s = ctx.enter_context(
            tc.tile_pool(name="psum_s", bufs=2, space="PSUM"))
        psum_g = ctx.enter_context(
            tc.tile_pool(name="psum_g", bufs=2, space="PSUM"))
        psum_u = ctx.enter_context(
            tc.tile_pool(name="psum_u", bufs=2, space="PSUM"))
        psum_y = ctx.enter_context(
            tc.tile_pool(name="psum_y", bufs=2, space="PSUM"))
        persist = ctx.enter_context(tc.tile_pool(name="persist", bufs=1))
        wstream = ctx.enter_context(tc.tile_pool(name="wstream", bufs=2))

        # ---------------- constants ----------------
        ident = consts.tile([128, 128], f32)
        masks.make_identity(nc, ident[:])
        if wdt != f32:
            ident_w = consts.tile([128, 128], wdt)
            nc.vector.tensor_copy(ident_w[:], ident[:])
        else:
            ident_w = ident

        ebias_b = consts.tile([128, E], f32)
        nc.scalar.dma_start(ebias_b[:], ebias[0:1, :].broadcast_to([128, E]))

        negbuf = consts.tile([128, E], f32)
        nc.gpsimd.memset(negbuf[:], -1e30)

        iota16_i = consts.tile([16, 16], dt.int32)
        nc.gpsimd.iota(iota16_i[:], pattern=[[16, 16]], base=0,
                       channel_multiplier=1)
        iota16 = consts.tile([16, 16], f32)
        nc.vector.tensor_copy(iota16[:], iota16_i[:])

        rwT_sb = consts.tile([128, DC, E], f32)
        nc.sync.dma_start(rwT_sb[:], rwT[:])

        # ---------------- P1: transpose own tokens -> xT [128, DC, TL] ----
        hst_cm = tc.tile_pool(name="hst", bufs=1)
        hstp = hst_cm.__enter__()
        xtw_cm = tc.tile_pool(name="xtw", bufs=1)
        xtwp = xtw_cm.__enter__()
        xtp_cm = tc.tile_pool(name="xtp", bufs=1)
        xtp = xtp_cm.__enter__()
        xT = xtp.tile([128, DC, TL], f32)
        for tt in range(TT):
            xrow = work.tile([128, D], f32, tag="xtile")
            nc.sync.dma_start(xrow[:], x_own[tt * 128:(tt + 1) * 128, :])
            for dc in range(DC):
                pt = psum_s.tile([128, 128], f32, tag="ps")
                nc.tensor.transpose(
                    pt[:], xrow[:, dc * 128:(dc + 1) * 128], ident[:])
                nc.vector.tensor_copy(
                    xT[:, dc, tt * 128:(tt + 1) * 128], pt[:])
        if wdt != f32:
            xTw = xtwp.tile([128, DC, TL], wdt)
            for dc in range(DC):
                nc.vector.tensor_copy(xTw[:, dc, :], xT[:, dc, :])
        else:
            xTw = xtwp.tile([128, DC, TL], f32)
            for dc in range(DC):
                nc.vector.tensor_copy(xTw[:, dc, :], xT[:, dc, :])

        # ---------------- P2: router on own tokens (fp32/exact) -----------
        for tt in range(TT):
            ps = psum_s.tile([128, E], f32, tag="ps")
            for dc in range(DC):
                nc.tensor.matmul(
                    ps[:], xT[:, dc, tt * 128:(tt + 1) * 128], rwT_sb[:, dc, :],
                    start=(dc == 0), stop=(dc == DC - 1))
            L = work.tile([128, E], f32, tag="rL")
            nc.vector.tensor_copy(L[:], ps[:])
            Ssig = work.tile([128, E], f32, tag="rS")
            nc.scalar.activation(Ssig[:], ps[:], Act.Sigmoid)
            Sb = work.tile([128, E], f32, tag="rSb")
            nc.vector.tensor_tensor(Sb[:], Ssig[:], ebias_b[:], op=Alu.add)

            # group score = top-2 sum per group = max over pair sums
            Sv = Sb[:].rearrange("p (g i) -> p g i", i=GS)
            gs = work.tile([128, NG], f32, tag="rGS")
            tmp = work.tile([128, NG], f32, tag="rtmp")
            nc.vector.tensor_tensor(gs[:], Sv[:, :, 0], Sv[:, :, 1], op=Alu.add)
            for (a, b) in [(0, 2), (0, 3), (1, 2), (1, 3), (2, 3)]:
                nc.vector.tensor_tensor(
                    tmp[:], Sv[:, :, a], Sv[:, :, b], op=Alu.add)
                nc.vector.tensor_tensor(gs[:], gs[:], tmp[:], op=Alu.max)

            m8g = work.tile([128, 8], f32, tag="rm8g")
            nc.vector.max(m8g[:], gs[:])
            gmask = work.tile([128, NG], f32, tag="rgm")
            nc.vector.tensor_scalar(
                gmask[:], gs[:], m8g[:, TKG - 1:TKG], None, op0=Alu.is_ge)

            emask = work.tile([128, E], f32, tag="rem")
            emv = emask[:].rearrange("p (g i) -> p g i", i=GS)
            for r in range(GS):
                nc.vector.tensor_copy(emv[:, :, r], gmask[:])

            # top-4 experts among unmasked, compared on exact logits
            emask8 = work.tile([128, E], dt.uint8, tag="rem8")
            nc.vector.tensor_copy(emask8[:], emask[:])
            ml = work.tile([128, E], f32, tag="rml")
            nc.vector.tensor_copy(ml[:], negbuf[:])
            nc.vector.copy_predicated(ml[:], emask8[:], L[:])
            m8e = work.tile([128, 8], f32, tag="rm8e")
            nc.vector.max(m8e[:], ml[:])
            sel = work.tile([128, E], f32, tag="rsel")
            nc.vector.tensor_scalar(
                sel[:], ml[:], m8e[:, K - 1:K], None, op0=Alu.is_ge)

            wm = work.tile([128, E], f32, tag="rwm")
            nc.vector.tensor_tensor(wm[:], Ssig[:], sel[:], op=Alu.mult)
            den = work.tile([128, 1], f32, tag="rden")
            nc.vector.tensor_reduce(
                den[:], wm[:], axis=mybir.AxisListType.X, op=Alu.add)
            nc.vector.tensor_scalar(den[:], den[:], 1e-20, None, op0=Alu.add)
            winv = work.tile([128, 1], f32, tag="rwinv")
            nc.vector.reciprocal(winv[:], den[:])

            # routed weights; w>0 encodes selection
            sw = work.tile([128, E], f32, tag="rsw")
            nc.vector.tensor_scalar(
                sw[:], wm[:], winv[:, 0:1], SCALE,
                op0=Alu.mult, op1=Alu.mult)
            nc.scalar.dma_start(selw_own[tt * 128:(tt + 1) * 128, :], sw[:])

        # ---------------- P3: AllGather routing (early issue) -------------
        if NOCC:
            for cc in range(NCORES):
                nc.scalar.dma_start(
                    selw_all[cc * TL:(cc + 1) * TL, :], selw_own[:])
        else:
            nc.gpsimd.collective_compute(
                "AllGather", Alu.bypass, replica_groups=groups,
                ins=[selw_own[:]], outs=[selw_all[:]])

        # ---------------- P8a: shared expert gate/up (covers AllGather) ---
        HsT = hstp.tile([128, MC, TL], wdt)

        # zero-row template for the bf16 [T, D/2] partials; the 64 row-chunk
        # writes are spread through the second p8a slice on both HWDGE
        # queues so they never head-of-line-block a weight stream
        zt = consts.tile([128, D], bf16)
        nc.gpsimd.memset(zt[:], 0.0)
        zjobs = [(partials[h], r) for r in range(NT) for h in range(2)]

        def emit_zeros(n):
            for _ in range(n):
                if not zjobs:
                    return
                par, r = zjobs.pop()
                eng = nc.sync if (len(zjobs) % 2) else nc.scalar
                eng.dma_start(par[r * 128:(r + 1) * 128, :], zt[:, 0:DH])

        def p8a(mc_lo, mc_hi, nz=0):
            for mc in range(mc_lo, mc_hi):
                sgu_t = wstream.tile([128, 2 * DC * 128], wdt, tag="wst")
                eng = nc.sync if mc % 2 == 0 else nc.scalar
                eng.dma_start(sgu_t[:], sgu[mc])
                emit_zeros(nz)
                pg = psum_g.tile([128, TL], f32, tag="pg")
                pu = psum_u.tile([128, TL], f32, tag="pu")
                for dc in range(DC):
                    nc.tensor.matmul(
                        pg[:], mm_cast(sgu_t[:, dc * 128:(dc + 1) * 128]),
                        mm_cast(xTw[:, dc, :]),
                        start=(dc == 0), stop=(dc == DC - 1))
                for dc in range(DC):
                    nc.tensor.matmul(
                        pu[:], mm_cast(
                            sgu_t[:, DC * 128 + dc * 128:
                                  DC * 128 + (dc + 1) * 128]),
                        mm_cast(xTw[:, dc, :]),
                        start=(dc == 0), stop=(dc == DC - 1))
                sig = work.tile([128, TL], f32, tag="ssig")
                nc.scalar.activation(sig[:], pg[:], Act.Sigmoid)
                sil = work.tile([128, TL], wdt, tag="ssil")
                nc.vector.tensor_tensor(sil[:], sig[:], pg[:], op=Alu.mult)
                nc.vector.tensor_tensor(
                    HsT[:, mc, :], sil[:], pu[:], op=Alu.mult)

        xtp_cm.__exit__(None, None, None)
        # first slice covers the AllGather latency; P4's small PE work is
        # interleaved next so its DVE/Pool chain overlaps the P8a remainder
        p8a(0, 8)

        # ---------------- P4: routing tables ------------------------------
        sloc_sb = consts.tile([E, EL], f32)
        nc.scalar.dma_start(sloc_sb[:], sloc[:])

        sgin_t = persist.tile([16, EL, T // 16], f32)
        sgin_w = persist.tile([16, EL, T // 16], f32)

        route_cm = tc.tile_pool(name="route", bufs=1)
        route = route_cm.__enter__()
        carry = persist.tile([EL, 1], f32)
        nc.gpsimd.memset(carry[:], 0.0)
        CH = 4
        CT = T // CH
        for q in range(CH):
            selwT = route.tile([E, CT // 128, 128], f32, tag="selwT")
            for j in range(CT // 128):
                tt = q * (CT // 128) + j
                swt = work.tile([128, E], f32, tag="swt")
                nc.scalar.dma_start(
                    swt[:], selw_all[tt * 128:(tt + 1) * 128, :])
                pt = psum_s.tile([E, 128], f32, tag="ps")
                nc.tensor.transpose(pt[:], swt[:], ident[:])
                nc.vector.tensor_copy(selwT[:, j, :], pt[:])

            SW_w = route.tile([EL, CT], f32, tag="SWw")
            for h in range(CT // 512):
                pswl = psum_g.tile([EL, 512], f32, tag="pg")
                nc.tensor.matmul(
                    pswl[:], sloc_sb[:], selwT[:, 4 * h:4 * (h + 1), :],
                    start=True, stop=True)
                nc.vector.tensor_copy(SW_w[:, h * 512:(h + 1) * 512], pswl[:])

            SW_sel = route.tile([EL, CT], f32, tag="SWsel")
            nc.vector.tensor_scalar(
                SW_sel[:], SW_w[:], 0.0, None, op0=Alu.is_gt)

            rank = route.tile([EL, CT], f32, tag="rank")
            nc.vector.tensor_tensor_scan(
                rank[:], SW_sel[:], SW_sel[:], carry[:, 0:1],
                op0=Alu.add, op1=Alu.bypass)
            nc.vector.tensor_copy(carry[:], rank[:, CT - 1:CT])

            fsel = route.tile([EL, CT], f32, tag="fsel")
            nc.vector.tensor_scalar(
                fsel[:], rank[:], float(C), None, op0=Alu.is_le)
            nc.vector.tensor_tensor(fsel[:], fsel[:], SW_sel[:], op=Alu.mult)

            iota_i = route.tile([EL, CT], dt.int32, tag="iotai")
            nc.gpsimd.iota(iota_i[:], pattern=[[1, CT]], base=1 + q * CT,
                           channel_multiplier=0)
            iota_f = route.tile([EL, CT], f32, tag="iotaf")
            nc.vector.tensor_copy(iota_f[:], iota_i[:])

            At = route.tile([EL, CT], f32, tag="At")
            nc.vector.tensor_tensor(At[:], fsel[:], iota_f[:], op=Alu.mult)
            nc.vector.tensor_scalar(At[:], At[:], 1.0, None, op0=Alu.subtract)

            fsel8 = route.tile([EL, CT], dt.uint8, tag="fsel8")
            nc.vector.tensor_copy(fsel8[:], fsel[:])
            Aw = route.tile([EL, CT], f32, tag="Aw")
            nc.gpsimd.memset(Aw[:], -1.0)
            nc.vector.copy_predicated(Aw[:], fsel8[:], SW_w[:])

            nc.scalar.dma_start(at_dram[:, q * CT:(q + 1) * CT], At[:])
            nc.scalar.dma_start(aw_dram[:, q * CT:(q + 1) * CT], Aw[:])

        for e in range(EL):
            nc.scalar.dma_start(
                sgin_t[:, e, :],
                at_dram[e].rearrange("(c b) -> b c", b=16))
            nc.scalar.dma_start(
                sgin_w[:, e, :],
                aw_dram[e].rearrange("(c b) -> b c", b=16))

        # per-expert compaction -> slot lists + weights
        idx16s = persist.tile([16, EL * 16], dt.int16)   # 16-row wrapped
        idx16 = persist.tile([128, EL * 16], dt.int16)   # replicated to 128
        w_col = persist.tile([128, 2 * EL], f32)

        # sparse_gather fills the tail beyond num_found with -1, which
        # dma_gather (transpose) tolerates and dma_scatter_add skips, so no
        # count masking is needed
        for e in range(EL):
            sgtok = work.tile([16, 16], f32, tag=f"sgtok{e}")
            nft = work.tile([1, 1], dt.uint32, tag=f"nft{e}")
            nc.gpsimd.sparse_gather(sgtok[:], sgin_t[:, e, :], num_found=nft[:])
            sgw = work.tile([16, 16], f32, tag=f"sgw{e}")
            nfw = work.tile([1, 1], dt.uint32, tag=f"nfw{e}")
            nc.gpsimd.sparse_gather(sgw[:], sgin_w[:, e, :], num_found=nfw[:])
            nc.vector.tensor_copy(idx16s[:, 16 * e:16 * (e + 1)], sgtok[:])

            ptw = psum_s.tile([16, 16], f32, tag="ps")
            nc.tensor.transpose(ptw[:], sgw[:], ident[:16, :16])
            wt16 = work.tile([16, 16], f32, tag=f"wt16{e}")
            nc.vector.tensor_copy(wt16[:], ptw[:])
            nc.scalar.dma_start(w_col[:, 2 * e:2 * e + 1], wt16[0:8, :])
            nc.scalar.dma_start(w_col[:, 2 * e + 1:2 * e + 2], wt16[8:16, :])

        # replicate the 16-row wrapped index block to all 128 partitions
        for r in range(8):
            nc.scalar.dma_start(idx16[16 * r:16 * (r + 1), :], idx16s[:])

        route_cm.__exit__(None, None, None)

        # cover P4's DVE/Pool/idx chain with more shared gate/up
        p8a(8, 14, nz=6)

        # ---------------- P5/P6: dispatch + gate/up for all experts -------
        ht_cm = tc.tile_pool(name="htp", bufs=1)
        htp = ht_cm.__enter__()
        HTall = htp.tile([128, EL, IC, C], wdt)
        with tc.tile_pool(name="dpXPT", bufs=2) as dpXPT:
            for e in range(EL):
                XPT = dpXPT.tile([128, DC, CP], wdt, tag="XPT")
                if wdt != f32:
                    nc.gpsimd.dma_gather(
                        XPT[:], x_full[:], idx16[:, 16 * e:16 * (e + 1)],
                        CP, CP, D, transpose=True, queue_num=0)
                else:
                    XP = dpXPT.tile([128, CP // 128, D], wdt, tag="XP")
                    nc.gpsimd.dma_gather(
                        XP[:], x_full[:], idx16[:, 16 * e:16 * (e + 1)],
                        CP, CP, D, queue_num=0)
                    for col in range(CP // 128):
                        for dc in range(DC):
                            ptx = psum_s.tile([128, 128], wdt, tag="ps")
                            nc.tensor.transpose(
                                ptx[:], XP[:, col, dc * 128:(dc + 1) * 128],
                                ident_w[:])
                            nc.vector.tensor_copy(
                                XPT[:, dc, col * 128:(col + 1) * 128], ptx[:])

                for ic in range(IC):
                    wgu_t = wstream.tile([128, 2 * DC * 128], wdt, tag="wst")
                    eng = nc.sync if ic % 2 == 0 else nc.scalar
                    eng.dma_start(wgu_t[:], wgu[e, ic])
                    emit_zeros(1)
                    pg = psum_g.tile([128, C], f32, tag="pg")
                    pu = psum_u.tile([128, C], f32, tag="pu")
                    for dc in range(DC):
                        nc.tensor.matmul(
                            pg[:], mm_cast(wgu_t[:, dc * 128:(dc + 1) * 128]),
                            mm_cast(XPT[:, dc, 0:C]),
                            start=(dc == 0), stop=(dc == DC - 1))
                    for dc in range(DC):
                        nc.tensor.matmul(
                            pu[:], mm_cast(
                                wgu_t[:, DC * 128 + dc * 128:
                                      DC * 128 + (dc + 1) * 128]),
                            mm_cast(XPT[:, dc, 0:C]),
                            start=(dc == 0), stop=(dc == DC - 1))
                    sig = work.tile([128, C], f32, tag="esig")
                    nc.scalar.activation(sig[:], pg[:], Act.Sigmoid)
                    sil = work.tile([128, C], wdt, tag="esil")
                    nc.vector.tensor_tensor(sil[:], sig[:], pg[:], op=Alu.mult)
                    nc.vector.tensor_tensor(
                        HTall[:, e, ic, :], sil[:], pu[:], op=Alu.mult)
                p8a(14 + e, min(15 + e, MC))

        # ---------------- P7: down-proj + scatter, column-halved ----------
        # first half's ReduceScatter overlaps P8b (shared down-proj) and the
        # second half's compute
        shr_sb = persist.tile([128, TT, DC4, 512], f32)

        def p8b():
            with tc.tile_pool(
                    name="sdpool", bufs=2 if wdt != f32 else 1) as sdpool:
                for dc4 in range(DC4):
                    sd_t = sdpool.tile([128, MC * 512], wdt, tag="wsd")
                    eng = nc.sync if dc4 % 2 == 0 else nc.scalar
                    eng.dma_start(sd_t[:], sd[dc4])
                    for tb in range(TT):
                        po = psum_y.tile([128, 512], f32, tag="py")
                        for mc in range(MC):
                            nc.tensor.matmul(
                                po[:],
                                mm_cast(HsT[:, mc, tb * 128:(tb + 1) * 128]),
                                mm_cast(sd_t[:, mc * 512:(mc + 1) * 512]),
                                start=(mc == 0), stop=(mc == MC - 1))
                        nc.vector.tensor_copy(shr_sb[:, tb, dc4, :], po[:])

        with tc.tile_pool(name="dpWD", bufs=2) as dpWD:
            for half in range(2):
                par = partials[half]
                for e in range(EL):
                    for q in range(2):
                        dc4 = half * 2 + q
                        wd_t = dpWD.tile([128, IC * 512], wdt, tag="wst3")
                        nc.sync.dma_start(wd_t[:], wd[e, dc4])
                        # slots 0:128
                        py0 = psum_y.tile([128, 512], f32, tag="py")
                        for ic in range(IC):
                            nc.tensor.matmul(
                                py0[:], mm_cast(HTall[:, e, ic, 0:128]),
                                mm_cast(wd_t[:, ic * 512:(ic + 1) * 512]),
                                start=(ic == 0), stop=(ic == IC - 1))
                        yw0 = work.tile([128, 512], bf16, tag="yw")
                        nc.vector.tensor_scalar(
                            yw0[:], py0[:], w_col[:, 2 * e:2 * e + 1], None,
                            op0=Alu.mult)
                        nc.gpsimd.dma_scatter_add(
                            par[:, q * 512:(q + 1) * 512],
                            yw0[:].rearrange("p (a f) -> p a f", a=1),
                            idx16[:, 16 * e:16 * e + 8],
                            128, 128, 512, elem_step=DH, queue_num=1)
                        # slots 128:160
                        py1 = psum_s.tile([32, 512], f32, tag="ps")
                        for ic in range(IC):
                            nc.tensor.matmul(
                                py1[:], mm_cast(HTall[:, e, ic, 128:C]),
                                mm_cast(wd_t[:, ic * 512:(ic + 1) * 512]),
                                start=(ic == 0), stop=(ic == IC - 1))
                        yw1 = work.tile([128, 512], bf16, tag="yw1")
                        nc.vector.tensor_scalar(
                            yw1[0:32, :], py1[:],
                            w_col[0:32, 2 * e + 1:2 * e + 2],
                            None, op0=Alu.mult)
                        nc.gpsimd.dma_scatter_add(
                            par[:, q * 512:(q + 1) * 512],
                            yw1[:].rearrange("p (a f) -> p a f", a=1),
                            idx16[:, 16 * e + 8:16 * e + 10],
                            32, 32, 512, elem_step=DH, queue_num=1)

                if NOCC:
                    nc.sync.dma_start(
                        rs_outs[half][:], partials[half][0:TL, :])
                else:
                    nc.gpsimd.collective_compute(
                        "ReduceScatter", Alu.add, replica_groups=groups,
                        ins=[partials[half][:]], outs=[rs_outs[half][:]])
                if half == 0:
                    p8b()

        ht_cm.__exit__(None, None, None)
        xtw_cm.__exit__(None, None, None)
        hst_cm.__exit__(None, None, None)

        # ---------------- P9: final add ----------------
        for half in range(2):
            for tb in range(TT):
                for q in range(2):
                    dc4 = half * 2 + q
                    rst = work.tile([128, 512], bf16, tag="rst")
                    nc.sync.dma_start(
                        rst[:], rs_outs[half][tb * 128:(tb + 1) * 128,
                                              q * 512:(q + 1) * 512])
                    fin = work.tile([128, 512], f32, tag="fin")
                    nc.vector.tensor_tensor(
                        fin[:], shr_sb[:, tb, dc4, :], rst[:], op=Alu.add)
                    nc.sync.dma_start(
                        out[tb * 128:(tb + 1) * 128,
                            dc4 * 512:(dc4 + 1) * 512], fin[:])

    nc.finalize()
    return nc


_NC_CACHE = {}


def get_nc(gemm_mode=None):
    gemm_mode = gemm_mode or GEMM_MODE
    key = (gemm_mode, NOCC)
    if key not in _NC_CACHE:
        _NC_CACHE[key] = _build(gemm_mode)
    return _NC_CACHE[key]


def make_in_maps(inputs, gemm_mode=None):
    """Shard full inputs into the 8 per-core input maps."""
    import ml_dtypes

    gemm_mode = gemm_mode or GEMM_MODE
    wnp = ml_dtypes.bfloat16 if gemm_mode == "bf16" else np.float32

    x = np.asarray(inputs["hidden_states"], np.float32).reshape(T, D)
    router_w = np.asarray(inputs["router_w"], np.float32)
    e_bias = np.asarray(inputs["e_bias"], np.float32).reshape(1, E)
    W_gate = np.asarray(inputs["W_gate"], np.float32)
    W_up = np.asarray(inputs["W_up"], np.float32)
    W_down = np.asarray(inputs["W_down"], np.float32)
    shared_gate = np.asarray(inputs["shared_gate"], np.float32)
    shared_up = np.asarray(inputs["shared_up"], np.float32)
    shared_down = np.asarray(inputs["shared_down"], np.float32)

    x_w = np.ascontiguousarray(x).astype(wnp)
    # rwTb[p, c, e] = router_w[e, c*128+p]
    rwTb = np.ascontiguousarray(
        router_w.reshape(E, DC, 128).transpose(2, 1, 0))

    # blocked layouts: every streaming DMA is one contiguous [128, N] block
    # wgub[e, ic, p, c*128+j] = W_gate[e][c*128+p, ic*128+j]; up in the
    # second half of the last axis
    wgb = (W_gate.reshape(E, DC, 128, IC, 128).transpose(0, 3, 2, 1, 4)
           .reshape(E, IC, 128, DC * 128))
    wub = (W_up.reshape(E, DC, 128, IC, 128).transpose(0, 3, 2, 1, 4)
           .reshape(E, IC, 128, DC * 128))
    wgub = np.ascontiguousarray(
        np.concatenate([wgb, wub], axis=3)).astype(wnp)
    # wdb[e, q, p, c*512+j] = W_down[e][c*128+p, q*512+j]
    wdb = np.ascontiguousarray(
        W_down.reshape(E, IC, 128, DC4, 512).transpose(0, 3, 2, 1, 4)
        .reshape(E, DC4, 128, IC * 512)).astype(wnp)
    # sgub[mc, p, c*128+j] = shared_gate[mc*128+j, c*128+p]; up in the
    # second half of the last axis
    sgb = (shared_gate.reshape(MC, 128, DC, 128).transpose(0, 3, 2, 1)
           .reshape(MC, 128, DC * 128))
    sub = (shared_up.reshape(MC, 128, DC, 128).transpose(0, 3, 2, 1)
           .reshape(MC, 128, DC * 128))
    sgub = np.ascontiguousarray(
        np.concatenate([sgb, sub], axis=2)).astype(wnp)
    # sdb[q, p, c*512+j] = shared_down[q*512+j, c*128+p]
    sdb = np.ascontiguousarray(
        shared_down.reshape(DC4, 512, MC, 128).transpose(0, 3, 2, 1)
        .reshape(DC4, 128, MC * 512)).astype(wnp)

    in_maps = []
    for c in range(NCORES):
        sl = np.zeros((E, EL), np.float32)
        for j in range(EL):
            sl[EL * c + j, j] = 1.0
        in_maps.append({
            "x_full": x_w,
            "x_own": np.ascontiguousarray(x[TL * c:TL * (c + 1)]),
            "rwT": rwTb,
            "ebias": e_bias,
            "sloc": sl,
            "wgu": np.ascontiguousarray(wgub[EL * c:EL * (c + 1)]),
            "wd": np.ascontiguousarray(wdb[EL * c:EL * (c + 1)]),
            "sgu": sgub,
            "sd": sdb,
        })
    return in_maps


def kernel(**inputs):
    from concourse.bass_utils import run_bass_kernel_spmd

    nc = get_nc()
    in_maps = make_in_maps(inputs)
    trace = bool(int(os.environ.get("BASS_MOE_TRACE", "0")))
    res = run_bass_kernel_spmd(
        nc, in_maps, core_ids=list(range(NCORES)), trace=trace)
    if trace and res.exec_time_ns is not None:
        print(f"HW exec time: {res.exec_time_ns} ns")
        kernel.last_exec_time_ns = res.exec_time_ns
    out = np.concatenate([res.results[c]["out"] for c in range(NCORES)], axis=0)
    return out.reshape(B, S, D)


kernel.last_exec_time_ns = None


# revision 27
# speedup vs baseline: 1.1789x; 1.1789x over previous
"""DeepseekV3 MoE layer on 8 Trainium2 NeuronCores (Bass/Tile).

Sharding:
  - Router: data-parallel (each core routes its own T/8=512 tokens, fp32,
    selection done on exact logits), then AllGather of per-token routed
    weights [T, E] (w>0 encodes selection) -> every core knows the routing.
  - Capacity ranks: per-expert running count over tokens via one DVE prefix
    scan; rank <= C survives (matches the reference's stable-sort capacity
    drop).
  - Routed experts: expert-parallel, 4 experts/core, bf16 GEMMs (router
    stays fp32).  Token rows are dma_gather'ed (transpose mode) straight
    into the [D-part, slot] layout, GEMM'd on the real capacity (160 of the
    256 padded slots), weighted, and dma_scatter_add'ed into bf16 [T, D/2]
    column-half partials.
  - Combine: two ReduceScatters (add, bf16), one per column half, so the
    first overlaps the second half's down-projection; each core adds its
    locally computed shared-expert MLP (kept in SBUF) and writes the fp32
    output slice.

Weights are pre-blocked host-side so every streaming DMA is a contiguous
[128, N] 2D transfer.  Small/bookkeeping DMAs ride the Activation HWDGE,
weight streams ride the SP HWDGE.

kernel(**inputs) takes the full unsharded inputs and returns the full
[B, S, D] output.  Self-contained: hardcodes all shapes.
"""

import os
import sys

for _p in ("/opt/trn_rl_repo", "/opt/pypackages"):
    if _p not in sys.path:
        sys.path.insert(0, _p)

import numpy as np

# ---------------------------------------------------------------- constants
B, S, D = 2, 2048, 2048
T = B * S                  # 4096 tokens
I = 1024                   # routed expert intermediate
E = 32                     # routed experts
K = 4                      # experts per token
NG = 8                     # groups
GS = E // NG               # experts per group = 4
TKG = 3                    # top-k groups
ISH = 2048                 # shared expert intermediate (I * n_shared)
SCALE = 2.5
C = 160                    # capacity = ceil(1.25 * T / E)
CP = 256                   # per-expert slot padding (128-aligned)
NCORES = 8
EL = E // NCORES           # local experts per core = 4
TL = T // NCORES           # local tokens per core = 512

# "bf16" | "f32" | "f32r" : dtype/mode of the heavy GEMMs (router stays f32)
GEMM_MODE = os.environ.get("BASS_MOE_GEMM_MODE", "bf16")
# ablation: 1 -> replace collectives with local copies (WRONG results;
# isolates the HW collective cost)
NOCC = bool(int(os.environ.get("BASS_MOE_NOCC", "0")))

DC = D // 128            # 16 d-chunks
IC = I // 128            # 8  i-chunks
MC = ISH // 128          # 16 shared-intermediate chunks
DC4 = D // 512           # 4  512-wide d-chunks
DH = D // 2              # column half width
TT = TL // 128           # 4 own-token tiles
NT = T // 128            # 32 all-token tiles


# ---------------------------------------------------------------- builder
def _build(gemm_mode: str):
    import concourse.bass as bass
    import concourse.bacc as bacc
    import concourse.mybir as mybir
    import concourse.tile as tile
    from concourse import masks
    from contextlib import ExitStack

    dt = mybir.dt
    Alu = mybir.AluOpType
    Act = mybir.ActivationFunctionType

    f32 = dt.float32
    bf16 = dt.bfloat16
    wdt = bf16 if gemm_mode == "bf16" else f32

    def mm_cast(ap):
        if gemm_mode == "f32r":
            return ap.bitcast(dt.float32r)
        return ap

    nc = bacc.Bacc(None, num_devices=NCORES, num_swdge_queues=2)
    groups = [list(range(NCORES))]

    # ---------------- I/O ----------------
    x_full = nc.dram_tensor("x_full", [T, D], wdt, kind="ExternalInput")
    x_own = nc.dram_tensor("x_own", [TL, D], f32, kind="ExternalInput")
    rwT = nc.dram_tensor("rwT", [128, DC, E], f32, kind="ExternalInput")
    ebias = nc.dram_tensor("ebias", [1, E], f32, kind="ExternalInput")
    sloc = nc.dram_tensor("sloc", [E, EL], f32, kind="ExternalInput")
    wgu = nc.dram_tensor("wgu", [EL, IC, 128, 2 * DC * 128], wdt,
                         kind="ExternalInput")
    wd = nc.dram_tensor("wd", [EL, DC4, 128, IC * 512], wdt,
                        kind="ExternalInput")
    sgu = nc.dram_tensor("sgu", [MC, 128, 2 * DC * 128], wdt,
                         kind="ExternalInput")
    sd = nc.dram_tensor("sd", [DC4, 128, MC * 512], wdt, kind="ExternalInput")
    out = nc.dram_tensor("out", [TL, D], f32, kind="ExternalOutput")

    # ---------------- internal DRAM ----------------
    selw_own = nc.dram_tensor("selw_own", [TL, E], f32)
    selw_all = nc.dram_tensor("selw_all", [T, E], f32, addr_space="Shared")
    partial0 = nc.dram_tensor("partial0", [T, DH], bf16)
    partial1 = nc.dram_tensor("partial1", [T, DH], bf16)
    rs_out0 = nc.dram_tensor("rs_out0", [TL, DH], bf16)
    rs_out1 = nc.dram_tensor("rs_out1", [TL, DH], bf16)
    idx_dram = nc.dram_tensor("idx_dram", [16, EL * 16], dt.int16)
    at_dram = nc.dram_tensor("at_dram", [EL, T], f32)
    nf_dram = nc.dram_tensor("nf_dram", [1, EL], f32)
    aw_dram = nc.dram_tensor("aw_dram", [EL, T], f32)

    partials = [partial0, partial1]
    rs_outs = [rs_out0, rs_out1]

    with tile.TileContext(nc) as tc, ExitStack() as ctx:
        consts = ctx.enter_context(tc.tile_pool(name="consts", bufs=1))
        work = ctx.enter_context(tc.tile_pool(name="work", bufs=2))
        psum_s = ctx.enter_context(
            tc.tile_pool(name="psum_s", bufs=2, space="PSUM"))
        psum_g = ctx.enter_context(
            tc.tile_pool(name="psum_g", bufs=2, space="PSUM"))
        psum_u = ctx.enter_context(
            tc.tile_pool(name="psum_u", bufs=2, space="PSUM"))
        psum_y = ctx.enter_context(
            tc.tile_pool(name="psum_y", bufs=2, space="PSUM"))
        persist = ctx.enter_context(tc.tile_pool(name="persist", bufs=1))
        wstream = ctx.enter_context(tc.tile_pool(name="wstream", bufs=2))

        # ---------------- constants ----------------
        ident = consts.tile([128, 128], f32)
        masks.make_identity(nc, ident[:])
        if wdt != f32:
            ident_w = consts.tile([128, 128], wdt)
            nc.vector.tensor_copy(ident_w[:], ident[:])
        else:
            ident_w = ident

        ebias_b = consts.tile([128, E], f32)
        nc.scalar.dma_start(ebias_b[:], ebias[0:1, :].broadcast_to([128, E]))

        negbuf = consts.tile([128, E], f32)
        nc.gpsimd.memset(negbuf[:], -1e30)

        iota16_i = consts.tile([16, 16], dt.int32)
        nc.gpsimd.iota(iota16_i[:], pattern=[[16, 16]], base=0,
                       channel_multiplier=1)
        iota16 = consts.tile([16, 16], f32)
        nc.vector.tensor_copy(iota16[:], iota16_i[:])

        rwT_sb = consts.tile([128, DC, E], f32)
        nc.sync.dma_start(rwT_sb[:], rwT[:])

        # ---------------- P1: transpose own tokens -> xT [128, DC, TL] ----
        hst_cm = tc.tile_pool(name="hst", bufs=1)
        hstp = hst_cm.__enter__()
        xtw_cm = tc.tile_pool(name="xtw", bufs=1)
        xtwp = xtw_cm.__enter__()
        xtp_cm = tc.tile_pool(name="xtp", bufs=1)
        xtp = xtp_cm.__enter__()
        xT = xtp.tile([128, DC, TL], f32)
        for tt in range(TT):
            xrow = work.tile([128, D], f32, tag="xtile")
            nc.sync.dma_start(xrow[:], x_own[tt * 128:(tt + 1) * 128, :])
            for dc in range(DC):
                pt = psum_s.tile([128, 128], f32, tag="ps")
                nc.tensor.transpose(
                    pt[:], xrow[:, dc * 128:(dc + 1) * 128], ident[:])
                nc.vector.tensor_copy(
                    xT[:, dc, tt * 128:(tt + 1) * 128], pt[:])
        if wdt != f32:
            xTw = xtwp.tile([128, DC, TL], wdt)
            for dc in range(DC):
                nc.vector.tensor_copy(xTw[:, dc, :], xT[:, dc, :])
        else:
            xTw = xtwp.tile([128, DC, TL], f32)
            for dc in range(DC):
                nc.vector.tensor_copy(xTw[:, dc, :], xT[:, dc, :])

        # ---------------- P2: router on own tokens (fp32/exact) -----------
        for tt in range(TT):
            ps = psum_s.tile([128, E], f32, tag="ps")
            for dc in range(DC):
                nc.tensor.matmul(
                    ps[:], xT[:, dc, tt * 128:(tt + 1) * 128], rwT_sb[:, dc, :],
                    start=(dc == 0), stop=(dc == DC - 1))
            L = work.tile([128, E], f32, tag="rL")
            nc.vector.tensor_copy(L[:], ps[:])
            Ssig = work.tile([128, E], f32, tag="rS")
            nc.scalar.activation(Ssig[:], ps[:], Act.Sigmoid)
            Sb = work.tile([128, E], f32, tag="rSb")
            nc.vector.tensor_tensor(Sb[:], Ssig[:], ebias_b[:], op=Alu.add)

            # group score = top-2 sum per group = max over pair sums
            Sv = Sb[:].rearrange("p (g i) -> p g i", i=GS)
            gs = work.tile([128, NG], f32, tag="rGS")
            tmp = work.tile([128, NG], f32, tag="rtmp")
            nc.vector.tensor_tensor(gs[:], Sv[:, :, 0], Sv[:, :, 1], op=Alu.add)
            for (a, b) in [(0, 2), (0, 3), (1, 2), (1, 3), (2, 3)]:
                nc.vector.tensor_tensor(
                    tmp[:], Sv[:, :, a], Sv[:, :, b], op=Alu.add)
                nc.vector.tensor_tensor(gs[:], gs[:], tmp[:], op=Alu.max)

            m8g = work.tile([128, 8], f32, tag="rm8g")
            nc.vector.max(m8g[:], gs[:])
            gmask = work.tile([128, NG], f32, tag="rgm")
            nc.vector.tensor_scalar(
                gmask[:], gs[:], m8g[:, TKG - 1:TKG], None, op0=Alu.is_ge)

            emask = work.tile([128, E], f32, tag="rem")
            emv = emask[:].rearrange("p (g i) -> p g i", i=GS)
            for r in range(GS):
                nc.vector.tensor_copy(emv[:, :, r], gmask[:])

            # top-4 experts among unmasked, compared on exact logits
            emask8 = work.tile([128, E], dt.uint8, tag="rem8")
            nc.vector.tensor_copy(emask8[:], emask[:])
            ml = work.tile([128, E], f32, tag="rml")
            nc.vector.tensor_copy(ml[:], negbuf[:])
            nc.vector.copy_predicated(ml[:], emask8[:], L[:])
            m8e = work.tile([128, 8], f32, tag="rm8e")
            nc.vector.max(m8e[:], ml[:])
            sel = work.tile([128, E], f32, tag="rsel")
            nc.vector.tensor_scalar(
                sel[:], ml[:], m8e[:, K - 1:K], None, op0=Alu.is_ge)

            wm = work.tile([128, E], f32, tag="rwm")
            nc.vector.tensor_tensor(wm[:], Ssig[:], sel[:], op=Alu.mult)
            den = work.tile([128, 1], f32, tag="rden")
            nc.vector.tensor_reduce(
                den[:], wm[:], axis=mybir.AxisListType.X, op=Alu.add)
            nc.vector.tensor_scalar(den[:], den[:], 1e-20, None, op0=Alu.add)
            winv = work.tile([128, 1], f32, tag="rwinv")
            nc.vector.reciprocal(winv[:], den[:])

            # routed weights; w>0 encodes selection
            sw = work.tile([128, E], f32, tag="rsw")
            nc.vector.tensor_scalar(
                sw[:], wm[:], winv[:, 0:1], SCALE,
                op0=Alu.mult, op1=Alu.mult)
            nc.scalar.dma_start(selw_own[tt * 128:(tt + 1) * 128, :], sw[:])

        # ---------------- P3: AllGather routing (early issue) -------------
        if NOCC:
            for cc in range(NCORES):
                nc.scalar.dma_start(
                    selw_all[cc * TL:(cc + 1) * TL, :], selw_own[:])
        else:
            nc.gpsimd.collective_compute(
                "AllGather", Alu.bypass, replica_groups=groups,
                ins=[selw_own[:]], outs=[selw_all[:]])

        # ---------------- P8a: shared expert gate/up (covers AllGather) ---
        HsT = hstp.tile([128, MC, TL], wdt)

        # zero-row template for the bf16 [T, D/2] partials; the 64 row-chunk
        # writes are spread through the second p8a slice on both HWDGE
        # queues so they never head-of-line-block a weight stream
        zt = consts.tile([128, D], bf16)
        nc.gpsimd.memset(zt[:], 0.0)
        zjobs = [(partials[h], r) for r in range(NT) for h in range(2)]

        def emit_zeros(n):
            for _ in range(n):
                if not zjobs:
                    return
                par, r = zjobs.pop()
                eng = nc.sync if (len(zjobs) % 2) else nc.scalar
                eng.dma_start(par[r * 128:(r + 1) * 128, :], zt[:, 0:DH])

        def p8a(mc_lo, mc_hi, nz=0):
            for mc in range(mc_lo, mc_hi):
                sgu_t = wstream.tile([128, 2 * DC * 128], wdt, tag="wst")
                eng = nc.sync if mc % 2 == 0 else nc.scalar
                eng.dma_start(sgu_t[:], sgu[mc])
                emit_zeros(nz)
                pg = psum_g.tile([128, TL], f32, tag="pg")
                pu = psum_u.tile([128, TL], f32, tag="pu")
                for dc in range(DC):
                    nc.tensor.matmul(
                        pg[:], mm_cast(sgu_t[:, dc * 128:(dc + 1) * 128]),
                        mm_cast(xTw[:, dc, :]),
                        start=(dc == 0), stop=(dc == DC - 1))
                for dc in range(DC):
                    nc.tensor.matmul(
                        pu[:], mm_cast(
                            sgu_t[:, DC * 128 + dc * 128:
                                  DC * 128 + (dc + 1) * 128]),
                        mm_cast(xTw[:, dc, :]),
                        start=(dc == 0), stop=(dc == DC - 1))
                sig = work.tile([128, TL], f32, tag="ssig")
                nc.scalar.activation(sig[:], pg[:], Act.Sigmoid)
                sil = work.tile([128, TL], wdt, tag="ssil")
                nc.vector.tensor_tensor(sil[:], sig[:], pg[:], op=Alu.mult)
                nc.vector.tensor_tensor(
                    HsT[:, mc, :], sil[:], pu[:], op=Alu.mult)

        xtp_cm.__exit__(None, None, None)
        # first slice covers the AllGather latency; P4's small PE work is
        # interleaved next so its DVE/Pool chain overlaps the P8a remainder
        p8a(0, 8)

        # ---------------- P4: routing tables ------------------------------
        sloc_sb = consts.tile([E, EL], f32)
        nc.scalar.dma_start(sloc_sb[:], sloc[:])

        sgin_t = persist.tile([16, EL, T // 16], f32)
        sgin_w = persist.tile([16, EL, T // 16], f32)

        route_cm = tc.tile_pool(name="route", bufs=1)
        route = route_cm.__enter__()
        carry = persist.tile([EL, 1], f32)
        nc.gpsimd.memset(carry[:], 0.0)
        CH = 4
        CT = T // CH
        for q in range(CH):
            selwT = route.tile([E, CT // 128, 128], f32, tag="selwT")
            for j in range(CT // 128):
                tt = q * (CT // 128) + j
                swt = work.tile([128, E], f32, tag="swt")
                nc.scalar.dma_start(
                    swt[:], selw_all[tt * 128:(tt + 1) * 128, :])
                pt = psum_s.tile([E, 128], f32, tag="ps")
                nc.tensor.transpose(pt[:], swt[:], ident[:])
                nc.vector.tensor_copy(selwT[:, j, :], pt[:])

            SW_w = route.tile([EL, CT], f32, tag="SWw")
            for h in range(CT // 512):
                pswl = psum_g.tile([EL, 512], f32, tag="pg")
                nc.tensor.matmul(
                    pswl[:], sloc_sb[:], selwT[:, 4 * h:4 * (h + 1), :],
                    start=True, stop=True)
                nc.vector.tensor_copy(SW_w[:, h * 512:(h + 1) * 512], pswl[:])

            SW_sel = route.tile([EL, CT], f32, tag="SWsel")
            nc.vector.tensor_scalar(
                SW_sel[:], SW_w[:], 0.0, None, op0=Alu.is_gt)

            rank = route.tile([EL, CT], f32, tag="rank")
            nc.vector.tensor_tensor_scan(
                rank[:], SW_sel[:], SW_sel[:], carry[:, 0:1],
                op0=Alu.add, op1=Alu.bypass)
            nc.vector.tensor_copy(carry[:], rank[:, CT - 1:CT])

            fsel = route.tile([EL, CT], f32, tag="fsel")
            nc.vector.tensor_scalar(
                fsel[:], rank[:], float(C), None, op0=Alu.is_le)
            nc.vector.tensor_tensor(fsel[:], fsel[:], SW_sel[:], op=Alu.mult)

            iota_i = route.tile([EL, CT], dt.int32, tag="iotai")
            nc.gpsimd.iota(iota_i[:], pattern=[[1, CT]], base=1 + q * CT,
                           channel_multiplier=0)
            iota_f = route.tile([EL, CT], f32, tag="iotaf")
            nc.vector.tensor_copy(iota_f[:], iota_i[:])

            At = route.tile([EL, CT], f32, tag="At")
            nc.vector.tensor_tensor(At[:], fsel[:], iota_f[:], op=Alu.mult)
            nc.vector.tensor_scalar(At[:], At[:], 1.0, None, op0=Alu.subtract)

            fsel8 = route.tile([EL, CT], dt.uint8, tag="fsel8")
            nc.vector.tensor_copy(fsel8[:], fsel[:])
            Aw = route.tile([EL, CT], f32, tag="Aw")
            nc.gpsimd.memset(Aw[:], -1.0)
            nc.vector.copy_predicated(Aw[:], fsel8[:], SW_w[:])

            nc.scalar.dma_start(at_dram[:, q * CT:(q + 1) * CT], At[:])
            nc.scalar.dma_start(aw_dram[:, q * CT:(q + 1) * CT], Aw[:])

        for e in range(EL):
            nc.scalar.dma_start(
                sgin_t[:, e, :],
                at_dram[e].rearrange("(c b) -> b c", b=16))
            nc.scalar.dma_start(
                sgin_w[:, e, :],
                aw_dram[e].rearrange("(c b) -> b c", b=16))

        # per-expert compaction -> slot lists + weights
        idx16s = persist.tile([16, EL * 16], dt.int16)   # 16-row wrapped
        idx16 = persist.tile([128, EL * 16], dt.int16)   # replicated to 128
        w_col = persist.tile([128, 2 * EL], f32)

        # sparse_gather fills the tail beyond num_found with -1, which
        # dma_gather (transpose) tolerates and dma_scatter_add skips, so no
        # count masking is needed
        for e in range(EL):
            sgtok = work.tile([16, 16], f32, tag=f"sgtok{e}")
            nft = work.tile([1, 1], dt.uint32, tag=f"nft{e}")
            nc.gpsimd.sparse_gather(sgtok[:], sgin_t[:, e, :], num_found=nft[:])
            sgw = work.tile([16, 16], f32, tag=f"sgw{e}")
            nfw = work.tile([1, 1], dt.uint32, tag=f"nfw{e}")
            nc.gpsimd.sparse_gather(sgw[:], sgin_w[:, e, :], num_found=nfw[:])
            nc.vector.tensor_copy(idx16s[:, 16 * e:16 * (e + 1)], sgtok[:])

            ptw = psum_s.tile([16, 16], f32, tag="ps")
            nc.tensor.transpose(ptw[:], sgw[:], ident[:16, :16])
            wt16 = work.tile([16, 16], f32, tag=f"wt16{e}")
            nc.vector.tensor_copy(wt16[:], ptw[:])
            nc.scalar.dma_start(w_col[:, 2 * e:2 * e + 1], wt16[0:8, :])
            nc.scalar.dma_start(w_col[:, 2 * e + 1:2 * e + 2], wt16[8:16, :])

        # replicate the 16-row wrapped index block to all 128 partitions
        for r in range(8):
            nc.scalar.dma_start(idx16[16 * r:16 * (r + 1), :], idx16s[:])

        route_cm.__exit__(None, None, None)

        # cover P4's DVE/Pool/idx chain with more shared gate/up
        p8a(8, 14, nz=6)

        # ---------------- P5/P6: dispatch + gate/up for all experts -------
        ht_cm = tc.tile_pool(name="htp", bufs=1)
        htp = ht_cm.__enter__()
        HTall = htp.tile([128, EL, IC, C], wdt)
        with tc.tile_pool(name="dpXPT", bufs=2) as dpXPT:
            for e in range(EL):
                XPT = dpXPT.tile([128, DC, CP], wdt, tag="XPT")
                if wdt != f32:
                    nc.gpsimd.dma_gather(
                        XPT[:], x_full[:], idx16[:, 16 * e:16 * (e + 1)],
                        CP, CP, D, transpose=True, queue_num=0)
                else:
                    XP = dpXPT.tile([128, CP // 128, D], wdt, tag="XP")
                    nc.gpsimd.dma_gather(
                        XP[:], x_full[:], idx16[:, 16 * e:16 * (e + 1)],
                        CP, CP, D, queue_num=0)
                    for col in range(CP // 128):
                        for dc in range(DC):
                            ptx = psum_s.tile([128, 128], wdt, tag="ps")
                            nc.tensor.transpose(
                                ptx[:], XP[:, col, dc * 128:(dc + 1) * 128],
                                ident_w[:])
                            nc.vector.tensor_copy(
                                XPT[:, dc, col * 128:(col + 1) * 128], ptx[:])

                for ic in range(IC):
                    wgu_t = wstream.tile([128, 2 * DC * 128], wdt, tag="wst")
                    eng = nc.sync if ic % 2 == 0 else nc.scalar
                    eng.dma_start(wgu_t[:], wgu[e, ic])
                    emit_zeros(1)
                    pg = psum_g.tile([128, C], f32, tag="pg")
                    pu = psum_u.tile([128, C], f32, tag="pu")
                    for dc in range(DC):
                        nc.tensor.matmul(
                            pg[:], mm_cast(wgu_t[:, dc * 128:(dc + 1) * 128]),
                            mm_cast(XPT[:, dc, 0:C]),
                            start=(dc == 0), stop=(dc == DC - 1))
                    for dc in range(DC):
                        nc.tensor.matmul(
                            pu[:], mm_cast(
                                wgu_t[:, DC * 128 + dc * 128:
                                      DC * 128 + (dc + 1) * 128]),
                            mm_cast(XPT[:, dc, 0:C]),
                            start=(dc == 0), stop=(dc == DC - 1))
                    sig = work.tile([128, C], f32, tag="esig")
                    nc.scalar.activation(sig[:], pg[:], Act.Sigmoid)
                    sil = work.tile([128, C], wdt, tag="esil")
                    nc.vector.tensor_tensor(sil[:], sig[:], pg[:], op=Alu.mult)
                    nc.vector.tensor_tensor(
                        HTall[:, e, ic, :], sil[:], pu[:], op=Alu.mult)
                p8a(14 + e, min(15 + e, MC))

        # ---------------- P7: down-proj + scatter, column-halved ----------
        # first half's ReduceScatter overlaps P8b (shared down-proj) and the
        # second half's compute
        shr_sb = persist.tile([128, TT, DC4, 512], f32)

        def p8b():
            with tc.tile_pool(
                    name="sdpool", bufs=2 if wdt != f32 else 1) as sdpool:
                for dc4 in range(DC4):
                    sd_t = sdpool.tile([128, MC * 512], wdt, tag="wsd")
                    eng = nc.sync if dc4 % 2 == 0 else nc.scalar
                    eng.dma_start(sd_t[:], sd[dc4])
                    for tb in range(TT):
                        po = psum_y.tile([128, 512], f32, tag="py")
                        for mc in range(MC):
                            nc.tensor.matmul(
                                po[:],
                                mm_cast(HsT[:, mc, tb * 128:(tb + 1) * 128]),
                                mm_cast(sd_t[:, mc * 512:(mc + 1) * 512]),
                                start=(mc == 0), stop=(mc == MC - 1))
                        nc.vector.tensor_copy(shr_sb[:, tb, dc4, :], po[:])

        with tc.tile_pool(name="dpWD", bufs=2) as dpWD:
            for half in range(2):
                par = partials[half]
                for e in range(EL):
                    for q in range(2):
                        dc4 = half * 2 + q
                        wd_t = dpWD.tile([128, IC * 512], wdt, tag="wst3")
                        nc.sync.dma_start(wd_t[:], wd[e, dc4])
                        # slots 0:128
                        py0 = psum_y.tile([128, 512], f32, tag="py")
                        for ic in range(IC):
                            nc.tensor.matmul(
                                py0[:], mm_cast(HTall[:, e, ic, 0:128]),
                                mm_cast(wd_t[:, ic * 512:(ic + 1) * 512]),
                                start=(ic == 0), stop=(ic == IC - 1))
                        yw0 = work.tile([128, 512], bf16, tag="yw")
                        nc.vector.tensor_scalar(
                            yw0[:], py0[:], w_col[:, 2 * e:2 * e + 1], None,
                            op0=Alu.mult)
                        nc.gpsimd.dma_scatter_add(
                            par[:, q * 512:(q + 1) * 512],
                            yw0[:].rearrange("p (a f) -> p a f", a=1),
                            idx16[:, 16 * e:16 * e + 8],
                            128, 128, 512, elem_step=DH, queue_num=1)
                        # slots 128:160
                        py1 = psum_s.tile([32, 512], f32, tag="ps")
                        for ic in range(IC):
                            nc.tensor.matmul(
                                py1[:], mm_cast(HTall[:, e, ic, 128:C]),
                                mm_cast(wd_t[:, ic * 512:(ic + 1) * 512]),
                                start=(ic == 0), stop=(ic == IC - 1))
                        yw1 = work.tile([128, 512], bf16, tag="yw1")
                        nc.vector.tensor_scalar(
                            yw1[0:32, :], py1[:],
                            w_col[0:32, 2 * e + 1:2 * e + 2],
                            None, op0=Alu.mult)
                        nc.gpsimd.dma_scatter_add(
                            par[:, q * 512:(q + 1) * 512],
                            yw1[:].rearrange("p (a f) -> p a f", a=1),
                            idx16[:, 16 * e + 8:16 * e + 10],
                            32, 32, 512, elem_step=DH, queue_num=1)

                if NOCC:
                    nc.sync.dma_start(
                        rs_outs[half][:], partials[half][0:TL, :])
                else:
                    nc.gpsimd.collective_compute(
                        "ReduceScatter", Alu.add, replica_groups=groups,
                        ins=[partials[half][:]], outs=[rs_outs[half][:]])
                if half == 0:
                    p8b()

        ht_cm.__exit__(None, None, None)
        xtw_cm.__exit__(None, None, None)
        hst_cm.__exit__(None, None, None)

        # ---------------- P9: final add ----------------
        for half in range(2):
            for tb in range(TT):
                for q in range(2):
                    dc4 = half * 2 + q
                    rst = work.tile([128, 512], bf16, tag="rst")
                    nc.sync.dma_start(
                        rst[:], rs_outs[half][tb * 128:(tb + 1) * 128,
                                              q * 512:(q + 1) * 512])
                    fin = work.tile([128, 512], f32, tag="fin")
                    nc.vector.tensor_tensor(
                        fin[:], shr_sb[:, tb, dc4, :], rst[:], op=Alu.add)
                    nc.sync.dma_start(
                        out[tb * 128:(tb + 1) * 128,
                            dc4 * 512:(dc4 + 1) * 512], fin[:])

    nc.finalize()
    return nc


_NC_CACHE = {}


def get_nc(gemm_mode=None):
    gemm_mode = gemm_mode or GEMM_MODE
    key = (gemm_mode, NOCC)
    if key not in _NC_CACHE:
        _NC_CACHE[key] = _build(gemm_mode)
    return _NC_CACHE[key]


def make_in_maps(inputs, gemm_mode=None):
    """Shard full inputs into the 8 per-core input maps."""
    import ml_dtypes

    gemm_mode = gemm_mode or GEMM_MODE
    wnp = ml_dtypes.bfloat16 if gemm_mode == "bf16" else np.float32

    x = np.asarray(inputs["hidden_states"], np.float32).reshape(T, D)
    router_w = np.asarray(inputs["router_w"], np.float32)
    e_bias = np.asarray(inputs["e_bias"], np.float32).reshape(1, E)
    W_gate = np.asarray(inputs["W_gate"], np.float32)
    W_up = np.asarray(inputs["W_up"], np.float32)
    W_down = np.asarray(inputs["W_down"], np.float32)
    shared_gate = np.asarray(inputs["shared_gate"], np.float32)
    shared_up = np.asarray(inputs["shared_up"], np.float32)
    shared_down = np.asarray(inputs["shared_down"], np.float32)

    x_w = np.ascontiguousarray(x).astype(wnp)
    # rwTb[p, c, e] = router_w[e, c*128+p]
    rwTb = np.ascontiguousarray(
        router_w.reshape(E, DC, 128).transpose(2, 1, 0))

    # blocked layouts: every streaming DMA is one contiguous [128, N] block
    # wgub[e, ic, p, c*128+j] = W_gate[e][c*128+p, ic*128+j]; up in the
    # second half of the last axis
    wgb = (W_gate.reshape(E, DC, 128, IC, 128).transpose(0, 3, 2, 1, 4)
           .reshape(E, IC, 128, DC * 128))
    wub = (W_up.reshape(E, DC, 128, IC, 128).transpose(0, 3, 2, 1, 4)
           .reshape(E, IC, 128, DC * 128))
    wgub = np.ascontiguousarray(
        np.concatenate([wgb, wub], axis=3)).astype(wnp)
    # wdb[e, q, p, c*512+j] = W_down[e][c*128+p, q*512+j]
    wdb = np.ascontiguousarray(
        W_down.reshape(E, IC, 128, DC4, 512).transpose(0, 3, 2, 1, 4)
        .reshape(E, DC4, 128, IC * 512)).astype(wnp)
    # sgub[mc, p, c*128+j] = shared_gate[mc*128+j, c*128+p]; up in the
    # second half of the last axis
    sgb = (shared_gate.reshape(MC, 128, DC, 128).transpose(0, 3, 2, 1)
           .reshape(MC, 128, DC * 128))
    sub = (shared_up.reshape(MC, 128, DC, 128).transpose(0, 3, 2, 1)
           .reshape(MC, 128, DC * 128))
    sgub = np.ascontiguousarray(
        np.concatenate([sgb, sub], axis=2)).astype(wnp)
    # sdb[q, p, c*512+j] = shared_down[q*512+j, c*128+p]
    sdb = np.ascontiguousarray(
        shared_down.reshape(DC4, 512, MC, 128).transpose(0, 3, 2, 1)
        .reshape(DC4, 128, MC * 512)).astype(wnp)

    in_maps = []
    for c in range(NCORES):
        sl = np.zeros((E, EL), np.float32)
        for j in range(EL):
            sl[EL * c + j, j] = 1.0
        in_maps.append({
            "x_full": x_w,
            "x_own": np.ascontiguousarray(x[TL * c:TL * (c + 1)]),
            "rwT": rwTb,
            "ebias": e_bias,
            "sloc": sl,
            "wgu": np.ascontiguousarray(wgub[EL * c:EL * (c + 1)]),
            "wd": np.ascontiguousarray(wdb[EL * c:EL * (c + 1)]),
            "sgu": sgub,
            "sd": sdb,
        })
    return in_maps


def kernel(**inputs):
    from concourse.bass_utils import run_bass_kernel_spmd

    nc = get_nc()
    in_maps = make_in_maps(inputs)
    trace = bool(int(os.environ.get("BASS_MOE_TRACE", "0")))
    res = run_bass_kernel_spmd(
        nc, in_maps, core_ids=list(range(NCORES)), trace=trace)
    if trace and res.exec_time_ns is not None:
        print(f"HW exec time: {res.exec_time_ns} ns")
        kernel.last_exec_time_ns = res.exec_time_ns
    out = np.concatenate([res.results[c]["out"] for c in range(NCORES)], axis=0)
    return out.reshape(B, S, D)


kernel.last_exec_time_ns = None
